# revision 1
# baseline (speedup 1.0000x reference)
"""Single-head causal attention kernel for Trainium2, 8-core data parallel.

Problem: x [8, 2048, 1024], Wk/Wq/Wv [64, 1024] ->
  out[b] = softmax(causal((x[b] @ Wq.T) @ (x[b] @ Wk.T).T / 8)) @ (x[b] @ Wv.T)

Sharding: one batch element per NeuronCore (data parallel across batch).

Per-core dataflow (all SBUF-resident, fp32):
  - host supplies xT = x[b].T [1024, 2048] so the embedding (contraction) dim
    lands on SBUF partitions directly; weights supplied pre-transposed and
    q/k fused: wqk = [Wq.T | Wk.T] [1024, 128].
  - qT/kT [64, 2048] computed with ONE packed matmul chain (stationary
    [128e, 128(q|k)] at full PE width); vT [64, 2048] separately.
  - v is re-transposed to natural [t_k, 64] via PE transpose, with a column
    of ones appended -> ve [t_k, 65]; the ones column makes the attention
    output matmul produce softmax row-sums for free.
  - scores are computed TRANSPOSED, sT[t_k, t_q] = k_j @ qT, so that
    P = exp(sT/8) needs no per-column bias (scores are bounded ~[-3, 4],
    max-subtraction is unnecessary in fp32) and P feeds the output matmul
    as the moving operand with no further transposes:
      out_psum[65, t_q] += ve_j.T @ P_j   (accumulated over key blocks j)
  - causal structure at 128-block granularity: only blocks t_k <= t_q are
    computed (136 of 256); the diagonal block is masked with a 0/1
    upper-triangular mask after exp.
  - device output is the unnormalized [65, 2048] (64 head dims + sums row);
    host divides by the sums row and transposes (0.26% of the FLOPs).
"""
import sys

for _p in ("/opt/trn_rl_repo",):
    if _p not in sys.path:
        sys.path.insert(0, _p)

import numpy as np
from contextlib import ExitStack

import concourse.bass as bass
import concourse.tile as tile
from concourse import bacc, mybir
from concourse.bass_utils import run_bass_kernel_spmd

FP = mybir.dt.float32
FPR = mybir.dt.float32r
B, T, E, H = 8, 2048, 1024, 64
NE = E // 128          # 8 e-tiles (contraction)
NT = T // 128          # 16 token tiles
CH = 512               # qkv column chunk (= one PSUM bank of fp32)
NC_CHUNKS = T // CH    # 4
SCALE = 1.0 / np.sqrt(H)  # 0.125

_CACHE = {}


def _to_fp32r(a):
    """Round fp32 to the fp32r grid (11 mantissa bits, round-to-nearest)."""
    u = np.ascontiguousarray(a, dtype=np.float32).view(np.uint32)
    u = (u + (((u >> 12) & 1) + 0x7FF)) & np.uint32(0xFFFFF000)
    return u.view(np.float32)


def _build_nc():
    nc = bacc.Bacc(None, target_bir_lowering=False, debug=False)

    xt_d = nc.dram_tensor("xt", [E, T], FPR, kind="ExternalInput")
    wqk_d = nc.dram_tensor("wqk", [E, 2 * H], FPR, kind="ExternalInput")
    wv_d = nc.dram_tensor("wv", [E, H], FPR, kind="ExternalInput")
    mask_d = nc.dram_tensor("mask", [128, 128], FPR, kind="ExternalInput")
    id_d = nc.dram_tensor("ident", [64, 64], FPR, kind="ExternalInput")
    ones_d = nc.dram_tensor("ones", [128, 1], FPR, kind="ExternalInput")
    out_d = nc.dram_tensor("out", [H + 1, T], FP, kind="ExternalOutput")

    with tile.TileContext(nc) as tc, ExitStack() as ctx:
        const = ctx.enter_context(tc.tile_pool(name="const", bufs=1))
        ve_pool = ctx.enter_context(tc.tile_pool(name="ve", bufs=NT))
        p_pool = ctx.enter_context(tc.tile_pool(name="pstripe", bufs=2))
        qk_psum = ctx.enter_context(
            tc.tile_pool(name="qk_ps", bufs=1, space=bass.MemorySpace.PSUM))
        vt_psum = ctx.enter_context(
            tc.tile_pool(name="vt_ps", bufs=1, space=bass.MemorySpace.PSUM))
        work_psum = ctx.enter_context(
            tc.tile_pool(name="work_ps", bufs=2, space=bass.MemorySpace.PSUM))
        out_psum = ctx.enter_context(
            tc.tile_pool(name="out_ps", bufs=1, space=bass.MemorySpace.PSUM))

        # ---- SBUF tensors ----
        # per-chunk tiles: Tile dependency tracking is tile-granular, so a
        # single big tile written by 4 chunk DMAs would stall every reader
        # until ALL chunks landed; separate tiles let compute start per chunk
        xts = [const.tile([128, NE * CH], FPR, name=f"xts{n}")
               for n in range(NC_CHUNKS)]
        wqk_sb = const.tile([128, NE * 2 * H], FPR)
        wv_sb = const.tile([128, NE * H], FPR)
        mask_sb = const.tile([128, 128], FPR)
        id_sb = const.tile([64, 64], FPR)
        qks = [const.tile([128, CH], FPR, name=f"qks{n}")
               for n in range(NC_CHUNKS)]             # rows 0:64 qT, 64:128 kT
        k_los = [const.tile([64, CH], FPR, name=f"klo{n}")
                 for n in range(NC_CHUNKS)]           # kT at partitions 0:64
        vTs = [const.tile([64, CH], FPR, name=f"vts{n}")
               for n in range(NC_CHUNKS)]
        out_sb = const.tile([H + 1, T], FP)
        ve = [ve_pool.tile([128, H + 1], FPR, tag="ve", name=f"ve{t}")
              for t in range(NT)]

        # ---- input DMAs ----
        nc.sync.dma_start(
            wqk_sb[:].rearrange("p (ne m) -> p ne m", m=2 * H),
            wqk_d.ap().rearrange("(ne p) m -> p ne m", p=128))
        nc.sync.dma_start(
            wv_sb[:].rearrange("p (ne m) -> p ne m", m=H),
            wv_d.ap().rearrange("(ne p) m -> p ne m", p=128))
        nc.sync.dma_start(mask_sb[:], mask_d.ap())
        nc.sync.dma_start(id_sb[:], id_d.ap())
        # x chunks, last column chunk first (attention unlocks high-j first)
        xt_in = xt_d.ap().rearrange("(ne p) t -> p ne t", p=128)

        for t in range(NT):
            nc.scalar.dma_start(ve[t][:, H:H + 1], ones_d.ap())

        out_ps = out_psum.tile([H + 1, T], FP)

        # 512-aligned piece list for the [j*128, T) column stripe of block j:
        # a leading 128-wide diagonal piece, then pieces up to the next
        # 512 boundary, then full 512s (PSUM-bank-aligned for out_ps).
        def stripe_pieces(j):
            pieces = [(j * 128, (j + 1) * 128)]
            c = (j + 1) * 128
            while c < T:
                e = min((c // CH + 1) * CH, T)
                pieces.append((c, e))
                c = e
            return pieces

        for n in range(NC_CHUNKS - 1, -1, -1):
            nc.sync.dma_start(
                xts[n][:].rearrange("p (ne t) -> p ne t", t=CH),
                xt_in[:, :, n * CH:(n + 1) * CH])
            # -- packed q|k projection for this column chunk --
            qk_ps = qk_psum.tile([128, CH], FP, tag="qk", name="qk_ps")
            for e in range(NE):
                nc.tensor.matmul(
                    qk_ps[:],
                    wqk_sb[:, bass.ts(e, 2 * H)],
                    xts[n][:, bass.ts(e, CH)],
                    start=(e == 0), stop=(e == NE - 1))
            nc.scalar.copy(qks[n][:], qk_ps[:])
            # move kT rows down to partitions 0:64 (partition remap via DMA)
            nc.scalar.dma_start(k_los[n][:], qks[n][64:128, :])
            # -- vT projection --
            vt_ps = vt_psum.tile([64, CH], FP, tag="vt", name="vt_ps")
            for e in range(NE):
                nc.tensor.matmul(
                    vt_ps[:],
                    wv_sb[:, bass.ts(e, H)],
                    xts[n][:, bass.ts(e, CH)],
                    start=(e == 0), stop=(e == NE - 1))
            nc.scalar.copy(vTs[n][:], vt_ps[:])
            # -- v natural tiles (PE transpose) + ones column --
            for t in range(4 * n, 4 * n + 4):
                tr_ps = work_psum.tile([128, CH], FPR, tag="work", name="tr_ps")
                nc.tensor.transpose(
                    tr_ps[:, 0:H], vTs[n][:, bass.ts(t - 4 * n, 128)], id_sb[:])
                nc.vector.tensor_copy(ve[t][:, 0:H], tr_ps[:, 0:H])

            # -- attention for key blocks j in this chunk (descending) --
            for j in range(4 * n + 3, 4 * n - 1, -1):
                pieces = stripe_pieces(j)
                stripe = p_pool.tile([128, T], FPR, tag="p")
                for (c0, c1) in pieces:
                    m = c0 // CH
                    s_ps = work_psum.tile([128, CH], FP, tag="work", name="s_ps")
                    nc.tensor.matmul(
                        s_ps[:, 0:c1 - c0],
                        k_los[j // 4][:, bass.ts(j % 4, 128)],
                        qks[m][0:64, c0 - m * CH: c1 - m * CH],
                        start=True, stop=True)
                    nc.scalar.activation(
                        stripe[:, c0 - j * 128: c1 - j * 128],
                        s_ps[:, 0:c1 - c0],
                        mybir.ActivationFunctionType.Exp,
                        scale=float(SCALE))
                # mask the diagonal block (upper-tri keep in [t_k, t_q])
                nc.vector.tensor_mul(
                    stripe[:, 0:128], stripe[:, 0:128], mask_sb[:])
                # out_psum[:, c0:c1] += ve_j.T @ P_j
                # start=True zeroes the target PSUM *bank*, so only the first
                # stripe to touch a bank (j % 4 == 3, descending) may set it;
                # stop marks the last write per bank (all j==0 pieces except
                # the diagonal one, whose bank is finished by the next piece).
                for (c0, c1) in pieces:
                    nc.tensor.matmul(
                        out_ps[:, c0:c1],
                        ve[j][:],
                        stripe[:, c0 - j * 128: c1 - j * 128],
                        start=(c0 == j * 128 and j % 4 == 3),
                        stop=(j == 0 and c0 != 0),
                        skip_group_check=True)

        for n in range(NC_CHUNKS):
            nc.vector.tensor_copy(out_sb[:, bass.ts(n, CH)],
                                  out_ps[:, bass.ts(n, CH)])
        nc.sync.dma_start(out_d.ap(), out_sb[:])

    nc.compile()
    return nc


def _get_nc():
    if "nc" not in _CACHE:
        _CACHE["nc"] = _build_nc()
    return _CACHE["nc"]


def kernel(x, Wk, Wq, Wv):
    x = np.ascontiguousarray(x, dtype=np.float32)
    assert x.shape == (B, T, E)
    nc = _get_nc()

    wqk = _to_fp32r(np.concatenate([Wq.T, Wk.T], axis=1))        # [E, 128]
    wv = _to_fp32r(Wv.T)                                          # [E, 64]
    mask = np.triu(np.ones((128, 128), dtype=np.float32))         # keep t_k <= t_q
    ident = np.eye(64, dtype=np.float32)
    ones = np.ones((128, 1), dtype=np.float32)

    in_maps = []
    for b in range(B):
        in_maps.append({
            "xt": _to_fp32r(x[b].T),
            "wqk": wqk,
            "wv": wv,
            "mask": mask,
            "ident": ident,
            "ones": ones,
        })

    res = run_bass_kernel_spmd(nc, in_maps, list(range(B)))
    out = np.empty((B, T, H), dtype=np.float32)
    for b in range(B):
        y = res.results[b]["out"]          # [65, T] unnormalized
        out[b] = (y[:H] / y[H:H + 1]).T
    return out


def run_traced(x, Wk, Wq, Wv):
    """Like kernel() but with NTFF profiling; returns (out, BassKernelResults)."""
    import types
    import antenv
    if "antenv.axon_hooks" not in sys.modules:
        hooks_mod = types.ModuleType("antenv.axon_hooks")
        _HOOK = [None]
        hooks_mod.set_axon_ntff_profile_hook = lambda h: _HOOK.__setitem__(0, h)
        hooks_mod.get_axon_ntff_profile_hook = lambda: _HOOK[0]
        sys.modules["antenv.axon_hooks"] = hooks_mod
        antenv.axon_hooks = hooks_mod
        from trn_agent_boot.trn_boot import _ntff_profile_via_ctypes
        hooks_mod.set_axon_ntff_profile_hook(
            _ntff_profile_via_ctypes("/opt/axon/libaxon_pjrt.so"))

    x = np.ascontiguousarray(x, dtype=np.float32)
    nc = _get_nc()
    wqk = _to_fp32r(np.concatenate([Wq.T, Wk.T], axis=1))
    wv = _to_fp32r(Wv.T)
    mask = np.triu(np.ones((128, 128), dtype=np.float32))
    ident = np.eye(64, dtype=np.float32)
    ones = np.ones((128, 1), dtype=np.float32)
    in_maps = [{
        "xt": _to_fp32r(x[b].T),
        "wqk": wqk, "wv": wv, "mask": mask, "ident": ident, "ones": ones,
    } for b in range(B)]
    res = run_bass_kernel_spmd(
        nc, in_maps, list(range(B)), trace=True, trace_cores=[0])
    out = np.empty((B, T, H), dtype=np.float32)
    for b in range(B):
        y = res.results[b]["out"]
        out[b] = (y[:H] / y[H:H + 1]).T
    return out, res



# revision 3
# speedup vs baseline: 1.7851x; 1.7851x over previous
"""Single-head causal attention kernel for Trainium2, 8-core data parallel.

Problem: x [8, 2048, 1024], Wk/Wq/Wv [64, 1024] ->
  out[b] = softmax(causal((x[b] @ Wq.T) @ (x[b] @ Wk.T).T / 8)) @ (x[b] @ Wv.T)

Sharding: one batch element per NeuronCore (data parallel across batch).

Per-core dataflow (bf16 PE operands, fp32 PSUM accumulation):
  - host supplies xT = x[b].T [1024, 2048] bf16 (PE streams bf16 at 1
    col/cycle vs 2 for fp32), weights pre-transposed/packed: wqk =
    [Wq.T | Wk.T] -> qT rows 0:64, kT rows 64:128 of the projection.
  - v is projected with COLUMN-TILED pairs (two concurrent matmuls on
    array col-halves, even e-tiles -> partitions 0:64, odd -> 64:128),
    halving v projection time; halves are folded after the PE transpose
    (partition offset becomes a column offset) by a DVE add.
  - a PE "swap" matmul (permutation stationary) produces [kT | qT]
    (halves swapped), so scores run as ROW-TILED pairs: two concurrent
    K=64 matmuls, tile (0,0) for even key blocks (kT from swap rows
    0:64 + qT original) and tile (64,0) for odd blocks (kT original +
    qT from swap rows 64:128) -- 2x score throughput.
  - scores are computed TRANSPOSED, sT[t_k, t_q] = k_j.T @ q, so that
    exp needs no max-subtraction (scores bounded ~[-4, 4] in fp32) and
    P feeds the output matmul as the moving operand:
      out_psum[65, t_q] += ve_j.T @ P_j  (ve = v natural + ones column
    making the softmax row-sums a free 65th output row).
  - causal structure: t_q is processed in 2 chunks of 1024; for chunk c
    only key blocks j with 128j < 1024(c+1) contribute, and each piece
    starts at column max(0, 128j-1024c); the diagonal 128-block is
    masked (0/1 upper-tri) after exp on DVE.
  - device output is unnormalized [65, 2048] fp32 (64 head dims + sums
    row); host divides by the sums row and transposes.
"""
import sys

for _p in ("/opt/trn_rl_repo",):
    if _p not in sys.path:
        sys.path.insert(0, _p)

import numpy as np
import ml_dtypes
from contextlib import ExitStack

import concourse.bass as bass
import concourse.tile as tile
from concourse import bacc, mybir
from concourse.bass_utils import run_bass_kernel_spmd

FP = mybir.dt.float32
BF = mybir.dt.bfloat16
B, T, E, H = 8, 2048, 1024, 64
NE = E // 128            # 8 e-tiles (contraction)
SUB = 512                # projection subchunk (= one PSUM bank of fp32)
NS = T // SUB            # 4
CH = 1024                # attention t_q chunk (2 PSUM banks)
NC_CH = T // CH          # 2
SCALE = 1.0 / np.sqrt(H)  # 0.125

_CACHE = {}


def _subs(n0):
    """512-aligned sub-ranges of [n0, CH)."""
    pieces = []
    a = n0
    while a < CH:
        b = min((a // SUB + 1) * SUB, CH)
        pieces.append((a, b))
        a = b
    return pieces


def _build_nc(do_compile=True):
    nc = bacc.Bacc(None, target_bir_lowering=False, debug=False)

    xt_d = nc.dram_tensor("xt", [E, T], BF, kind="ExternalInput")
    wqk_d = nc.dram_tensor("wqk", [128, NE * 128], BF, kind="ExternalInput")
    wv_d = nc.dram_tensor("wv", [128, NE * H], BF, kind="ExternalInput")
    cst_d = nc.dram_tensor("cst", [128, 3 * 128], BF, kind="ExternalInput")
    out_d = nc.dram_tensor("out", [H + 1, T], FP, kind="ExternalOutput")

    with tile.TileContext(nc) as tc, ExitStack() as ctx:
        const = ctx.enter_context(tc.tile_pool(name="const", bufs=1))
        stripes = ctx.enter_context(tc.tile_pool(name="stripe", bufs=4))
        pp = ctx.enter_context(
            tc.tile_pool(name="pp", bufs=2, space=bass.MemorySpace.PSUM))
        sps = ctx.enter_context(
            tc.tile_pool(name="sps", bufs=2, space=bass.MemorySpace.PSUM))
        ops = ctx.enter_context(
            tc.tile_pool(name="ops", bufs=1, space=bass.MemorySpace.PSUM))

        # ---- SBUF tensors ----
        wqk_sb = const.tile([128, NE * 128], BF)
        wv_sb = const.tile([128, NE * H], BF)
        cst_sb = const.tile([128, 3 * 128], BF)
        perm = cst_sb[:, 0:128]
        ident = cst_sb[:, 128:256]
        mask = cst_sb[:, 256:384]
        xts = [[const.tile([128, SUB], BF, name=f"x{s}e{e}")
                for e in range(NE)] for s in range(NS)]
        qks = [const.tile([128, SUB], BF, name=f"qks{s}") for s in range(NS)]
        qsw = [const.tile([128, SUB], BF, name=f"qsw{s}") for s in range(NS)]
        vsb = [const.tile([128, SUB], BF, name=f"vsb{s}") for s in range(NS)]
        ve = [const.tile([128, H + 1], BF, name=f"ve{t}") for t in range(T // 128)]
        outc = [const.tile([H + 1, CH], FP, name=f"outc{c}") for c in range(NC_CH)]

        # ---- input DMAs (weights/constants first, then x pieces in use order)
        nc.sync.dma_start(wqk_sb[:], wqk_d.ap())
        nc.sync.dma_start(wv_sb[:], wv_d.ap())
        nc.sync.dma_start(cst_sb[:], cst_d.ap())
        xt_ap = xt_d.ap()
        for s in range(NS):
            for e in range(NE):
                nc.sync.dma_start(
                    xts[s][e][:],
                    xt_ap[128 * e:128 * (e + 1), SUB * s:SUB * (s + 1)])

        for t in range(T // 128):
            nc.vector.memset(ve[t][:, H:H + 1], 1.0)

        out_ps = [None, None]
        out_ps[0] = ops.tile([H + 1, CH], FP, tag="ops", name="out_ps0")

        # ---- PE warmup: junk matmuls to lift the HAM clock gate while the
        # first x pieces stream in (results are overwritten by start=True).
        for i in range(16):
            nc.tensor.matmul(
                out_ps[0][:, 0:SUB], wqk_sb[:, 0:H + 1], wqk_sb[:, 0:SUB],
                start=True, stop=True, skip_group_check=True)

        def proj(s):
            qk_ps = pp.tile([128, SUB], FP, tag="pp", name="qk_ps")
            for e in range(NE):
                nc.tensor.matmul(
                    qk_ps[:], wqk_sb[:, 128 * e:128 * (e + 1)], xts[s][e][:],
                    start=(e == 0), stop=(e == NE - 1))
            nc.vector.tensor_copy(qks[s][:], qk_ps[:])
            sw_ps = pp.tile([128, SUB], FP, tag="pp", name="sw_ps")
            nc.tensor.matmul(sw_ps[:], perm, qks[s][:], start=True, stop=True)
            nc.vector.tensor_copy(qsw[s][:], sw_ps[:])
            v_ps = pp.tile([128, SUB], FP, tag="pp", name="v_ps")
            for ep in range(4):
                nc.tensor.matmul(
                    v_ps[0:64, :], wv_sb[:, H * 2 * ep:H * (2 * ep + 1)],
                    xts[s][2 * ep][:],
                    start=(ep == 0), stop=(ep == 3), skip_group_check=True)
                nc.tensor.matmul(
                    v_ps[64:128, :], wv_sb[:, H * (2 * ep + 1):H * (2 * ep + 2)],
                    xts[s][2 * ep + 1][:],
                    start=(ep == 0), stop=(ep == 3), skip_group_check=True,
                    tile_position=(0, 64))
            nc.vector.tensor_copy(vsb[s][:], v_ps[:])
            for ti in range(4):
                t = 4 * s + ti
                tr_ps = pp.tile([128, 128], BF, tag="pp", name="tr_ps")
                nc.tensor.transpose(
                    tr_ps[:], vsb[s][:, 128 * ti:128 * (ti + 1)], ident)
                # DVE may read only ONE operand from PSUM per instruction
                nc.vector.tensor_copy(ve[t][:, 0:H], tr_ps[:, 0:64])
                nc.vector.tensor_add(ve[t][:, 0:H], ve[t][:, 0:H],
                                     tr_ps[:, 64:128])

        def scores(c, j, hi):
            n0 = max(0, 128 * j - CH * c)
            sj, bo = j // 4, 128 * (j % 4)
            s_ps = sps.tile([128, CH], FP, tag="sps", name="s_ps")
            for (a, b) in _subs(n0):
                m = (CH * c + a) // SUB
                off = CH * c + a - SUB * m
                if hi:
                    nc.tensor.matmul(
                        s_ps[:, a:b], qks[sj][64:128, bo:bo + 128],
                        qsw[m][64:128, off:off + (b - a)],
                        start=True, stop=True)
                else:
                    nc.tensor.matmul(
                        s_ps[:, a:b], qsw[sj][0:64, bo:bo + 128],
                        qks[m][0:64, off:off + (b - a)],
                        start=True, stop=True)
            return s_ps, n0

        def exp_mask(c, j, s_ps, n0):
            stripe = stripes.tile([128, CH], BF, tag="stripe", name="stripe")
            nc.scalar.activation(
                stripe[:, 0:CH - n0], s_ps[:, n0:CH],
                mybir.ActivationFunctionType.Exp, scale=float(SCALE))
            if 128 * j >= CH * c:  # diagonal block is in this chunk
                nc.vector.tensor_mul(stripe[:, 0:128], stripe[:, 0:128], mask)
            return stripe

        def attn_out(c, j, stripe, n0):
            nj = 8 * c + 7  # last key block for this chunk
            for (a, b) in _subs(n0):
                bank_last = (nj - 4) if a < SUB else nj
                nc.tensor.matmul(
                    out_ps[c][:, a:b], ve[j][:], stripe[:, a - n0:b - n0],
                    start=(j == 0), stop=(j == bank_last),
                    skip_group_check=True)

        # ---- main schedule ----
        proj(0)
        proj(1)

        def attn_chunk(c, filler):
            npair = 4 * (c + 1)
            prev = None
            for p in range(npair):
                ja, jb = 2 * p, 2 * p + 1
                sa, n0a = scores(c, ja, hi=False)
                sb_, n0b = scores(c, jb, hi=True)
                if p == 0 and filler:
                    filler[0]()
                ea = exp_mask(c, ja, sa, n0a)
                eb = exp_mask(c, jb, sb_, n0b)
                if prev is not None:
                    attn_out(c, prev[0], prev[1], prev[2])
                    attn_out(c, prev[3], prev[4], prev[5])
                if p == 1 and len(filler) > 1:
                    filler[1]()
                prev = (ja, ea, n0a, jb, eb, n0b)
            attn_out(c, prev[0], prev[1], prev[2])
            attn_out(c, prev[3], prev[4], prev[5])
            nc.vector.tensor_copy(outc[c][:], out_ps[c][:])
            nc.sync.dma_start(out_d.ap()[:, CH * c:CH * (c + 1)], outc[c][:])

        attn_chunk(0, [lambda: proj(2), lambda: proj(3)])
        out_ps[1] = ops.tile([H + 1, CH], FP, tag="ops", name="out_ps1")
        attn_chunk(1, [])

    if do_compile:
        nc.compile()
    return nc


def _get_nc():
    if "nc" not in _CACHE:
        _CACHE["nc"] = _build_nc()
    return _CACHE["nc"]


def _host_inputs(x, Wk, Wq, Wv):
    bf = ml_dtypes.bfloat16
    wqkT = np.concatenate([Wq.T, Wk.T], axis=1)            # [E, 128]
    wqk = np.ascontiguousarray(
        wqkT.reshape(NE, 128, 128).transpose(1, 0, 2).reshape(128, NE * 128)
    ).astype(bf)
    wvT = Wv.T                                             # [E, 64]
    wv = np.ascontiguousarray(
        wvT.reshape(NE, 128, H).transpose(1, 0, 2).reshape(128, NE * H)
    ).astype(bf)
    z = np.zeros((64, 64), np.float32)
    i64 = np.eye(64, dtype=np.float32)
    permh = np.block([[z, i64], [i64, z]])
    identh = np.eye(128, dtype=np.float32)
    maskh = np.triu(np.ones((128, 128), np.float32))       # keep t_k <= t_q
    cst = np.concatenate([permh, identh, maskh], axis=1).astype(bf)
    return wqk, wv, cst


def kernel(x, Wk, Wq, Wv):
    x = np.ascontiguousarray(x, dtype=np.float32)
    assert x.shape == (B, T, E)
    nc = _get_nc()
    wqk, wv, cst = _host_inputs(x, Wk, Wq, Wv)
    bf = ml_dtypes.bfloat16
    in_maps = [{
        "xt": np.ascontiguousarray(x[b].T).astype(bf),
        "wqk": wqk, "wv": wv, "cst": cst,
    } for b in range(B)]

    res = run_bass_kernel_spmd(nc, in_maps, list(range(B)))
    out = np.empty((B, T, H), dtype=np.float32)
    for b in range(B):
        y = res.results[b]["out"]          # [65, T] unnormalized
        out[b] = (y[:H] / y[H:H + 1]).T
    return out


def run_traced(x, Wk, Wq, Wv):
    """Like kernel() but with NTFF profiling; returns (out, BassKernelResults)."""
    import types
    import antenv
    if "antenv.axon_hooks" not in sys.modules:
        hooks_mod = types.ModuleType("antenv.axon_hooks")
        _HOOK = [None]
        hooks_mod.set_axon_ntff_profile_hook = lambda h: _HOOK.__setitem__(0, h)
        hooks_mod.get_axon_ntff_profile_hook = lambda: _HOOK[0]
        sys.modules["antenv.axon_hooks"] = hooks_mod
        antenv.axon_hooks = hooks_mod
        from trn_agent_boot.trn_boot import _ntff_profile_via_ctypes
        hooks_mod.set_axon_ntff_profile_hook(
            _ntff_profile_via_ctypes("/opt/axon/libaxon_pjrt.so"))

    x = np.ascontiguousarray(x, dtype=np.float32)
    nc = _get_nc()
    wqk, wv, cst = _host_inputs(x, Wk, Wq, Wv)
    bf = ml_dtypes.bfloat16
    in_maps = [{
        "xt": np.ascontiguousarray(x[b].T).astype(bf),
        "wqk": wqk, "wv": wv, "cst": cst,
    } for b in range(B)]
    res = run_bass_kernel_spmd(
        nc, in_maps, list(range(B)), trace=True, trace_cores=[0])
    out = np.empty((B, T, H), dtype=np.float32)
    for b in range(B):
        y = res.results[b]["out"]
        out[b] = (y[:H] / y[H:H + 1]).T
    return out, res
